# revision 8
# baseline (speedup 1.0000x reference)
"""DECOLLE network (2 CUBA-LIF layers + linear readouts) on 8 trn2 NeuronCores.

Sharding: data-parallel over batch (32 -> 4 per core), weights replicated.

v2 design:
- GEMMs in fp16 hi/lo weight splitting (w = hi + lo, both fp16): spike inputs
  are exactly representable, products accumulate in fp32 PSUM, total weight
  quantization error ~2^-22 relative (fp32-quality), at fp16 matmul speed
  (1 cyc/col vs fp32's 4).
- Current recurrence via hardware tensor_tensor_scan along t (per hc,b).
- Voltage loop: 2 dependent scalar_tensor_tensor per step on fully
  contiguous [128, 32] tiles (state + t-major cur/vtmp rings).
- Spikes: bulk is_ge producing both the fp16 GEMM operand and the fp32
  output copy.
- v output: ACT transposes t-major vtmp into (hc,b,t) staging for DMA.
- Readouts accumulated inline per chunk, PSUM bank shared with z1 rotation.
"""

import sys

sys.path.insert(0, "/opt/trn_rl_repo")

import numpy as np

import concourse.bass as bass
import concourse.tile as tile
from concourse import bacc, mybir
from concourse.bass_utils import run_bass_kernel_spmd

F32 = mybir.dt.float32
F16 = mybir.dt.float16
OP = mybir.AluOpType

THRESH = 1.25
CUR_DECAY = 0.25
VOLT_DECAY = 0.03

B, F, T = 32, 512, 256
H, OUT = 1024, 128
N_CORES = 8
BL = B // N_CORES          # 4 batches per core
TC = 64                    # time chunk
NCH = T // TC              # 4 chunks
HC = H // 128              # 8 h chunks
FC = F // 128              # 4 f chunks
NST = HC * BL              # 32 state lanes per partition-row


def build_nc():
    nc = bacc.Bacc(None)

    x16 = nc.declare_dram_parameter("x16", [BL, F, T], F16, isOutput=False)
    w1hi = nc.declare_dram_parameter("w1hi", [F, H], F16, isOutput=False)
    w1lo = nc.declare_dram_parameter("w1lo", [F, H], F16, isOutput=False)
    w2hi = nc.declare_dram_parameter("w2hi", [H, H], F16, isOutput=False)
    w2lo = nc.declare_dram_parameter("w2lo", [H, H], F16, isOutput=False)
    r1hi = nc.declare_dram_parameter("r1hi", [H, OUT], F16, isOutput=False)
    r1lo = nc.declare_dram_parameter("r1lo", [H, OUT], F16, isOutput=False)
    r2hi = nc.declare_dram_parameter("r2hi", [H, OUT], F16, isOutput=False)
    r2lo = nc.declare_dram_parameter("r2lo", [H, OUT], F16, isOutput=False)

    s1_d = nc.declare_dram_parameter("s1", [BL, H, T], F32, isOutput=True)
    v1_d = nc.declare_dram_parameter("v1", [BL, H, T], F32, isOutput=True)
    s2_d = nc.declare_dram_parameter("s2", [BL, H, T], F32, isOutput=True)
    v2_d = nc.declare_dram_parameter("v2", [BL, H, T], F32, isOutput=True)
    r1_d = nc.declare_dram_parameter("r1", [BL, OUT, T], F32, isOutput=True)
    r2_d = nc.declare_dram_parameter("r2", [BL, OUT, T], F32, isOutput=True)

    spike_r = x16[:].rearrange("b (fc p) t -> fc p b t", p=128)
    s1_r = s1_d[:].rearrange("b (hc p) t -> p hc b t", p=128)
    v1_r = v1_d[:].rearrange("b (hc p) t -> p hc b t", p=128)
    s2_r = s2_d[:].rearrange("b (hc p) t -> p hc b t", p=128)
    v2_r = v2_d[:].rearrange("b (hc p) t -> p hc b t", p=128)
    r1_r = r1_d[:].rearrange("b o t -> o b t")
    r2_r = r2_d[:].rearrange("b o t -> o b t")

    w1hi_r = w1hi[:].rearrange("(fc p) h -> p fc h", p=128)
    w1lo_r = w1lo[:].rearrange("(fc p) h -> p fc h", p=128)
    w2hi_r = w2hi[:].rearrange("(kc p) h -> p kc h", p=128)
    w2lo_r = w2lo[:].rearrange("(kc p) h -> p kc h", p=128)
    r1hi_r = r1hi[:].rearrange("(kc p) o -> p kc o", p=128)
    r1lo_r = r1lo[:].rearrange("(kc p) o -> p kc o", p=128)
    r2hi_r = r2hi[:].rearrange("(kc p) o -> p kc o", p=128)
    r2lo_r = r2lo[:].rearrange("(kc p) o -> p kc o", p=128)

    with tile.TileContext(nc) as tc:
        with (
            tc.tile_pool(name="wsb", bufs=1) as wsb,
            tc.tile_pool(name="xsb", bufs=2) as xsb,
            tc.tile_pool(name="cursb", bufs=2) as cursb,
            tc.tile_pool(name="vtsb", bufs=1) as vtsb,
            tc.tile_pool(name="vosb", bufs=1) as vosb,
            tc.tile_pool(name="ssb", bufs=1) as ssb,
            tc.tile_pool(name="sfsb", bufs=2) as sfsb,
            tc.tile_pool(name="rsb", bufs=1) as rsb,
            tc.tile_pool(name="psum1", bufs=1, space="PSUM") as psum1,
            tc.tile_pool(name="psum2", bufs=1, space="PSUM") as psum2,
        ):
            # ---- x chunk 0 first on the SP ring so PE can start early;
            #      weights go on the ACT ring (separate FIFO) ----
            x_first = []
            for fc in range(FC):
                x = xsb.tile([128, BL, TC], F16, tag=f"x{fc}", name=f"x{fc}_0")
                nc.sync.dma_start(out=x, in_=spike_r[fc][:, :, 0:TC])
                x_first.append(x)

            w1h = wsb.tile([128, FC, H], F16, name="w1h")
            nc.scalar.dma_start(out=w1h, in_=w1hi_r)
            w1l = wsb.tile([128, FC, H], F16, name="w1l")
            nc.scalar.dma_start(out=w1l, in_=w1lo_r)
            w2h = wsb.tile([128, HC, H], F16, name="w2h")
            nc.scalar.dma_start(out=w2h, in_=w2hi_r)
            w2l = wsb.tile([128, HC, H], F16, name="w2l")
            nc.scalar.dma_start(out=w2l, in_=w2lo_r)
            r1h = wsb.tile([128, HC, OUT], F16, name="r1h")
            nc.scalar.dma_start(out=r1h, in_=r1hi_r)
            r1l = wsb.tile([128, HC, OUT], F16, name="r1l")
            nc.scalar.dma_start(out=r1l, in_=r1lo_r)
            r2h = wsb.tile([128, HC, OUT], F16, name="r2h")
            nc.scalar.dma_start(out=r2h, in_=r2hi_r)
            r2l = wsb.tile([128, HC, OUT], F16, name="r2l")
            nc.scalar.dma_start(out=r2l, in_=r2lo_r)

            c075 = wsb.tile([128, TC], F32, name="c075")
            nc.vector.memset(c075, 1.0 - CUR_DECAY)
            v1st = wsb.tile([128, NST], F32, name="v1st")
            nc.vector.memset(v1st, 0.0)
            v2st = wsb.tile([128, NST], F32, name="v2st")
            nc.vector.memset(v2st, 0.0)

            s1_16 = ssb.tile([128, HC, BL, T], F16, name="s1_16")
            s2_16 = ssb.tile([128, HC, BL, T], F16, name="s2_16")
            r1sb = rsb.tile([128, BL, T], F32, name="r1sb")
            r2sb = rsb.tile([128, BL, T], F32, name="r2sb")

            def layer(c, tsl, x_tiles, wh, wl, nk, cur_tag, prev_cur, vst,
                      vt_tag, vo_tag, sf_tag, s16, ps, ps_tag, s_r, v_r):
                """One layer's chunk: GEMM -> scan -> voltage -> spikes -> DMA.

                x_tiles: list of rhs tiles (or callable kc -> AP) with [128, BL, TC]
                wh/wl: weight tiles [128, nk, H] (hi / lo)
                Returns cur ring tile (for next chunk's scan initial).
                """
                zp = []
                for g in range(HC // 2):
                    p = ps.tile([128, 2, BL, TC], F32, tag=f"{ps_tag}{g}",
                                name=f"z{ps_tag}{g}_{c}")
                    zp.append(p)
                for g in range(HC // 2):
                    for i in range(2):
                        hc = 2 * g + i
                        hsl = slice(hc * 128, (hc + 1) * 128)
                        for kc in range(nk):
                            nc.tensor.matmul(
                                zp[g][:, i], wh[:, kc, hsl], x_tiles(kc),
                                start=(kc == 0), stop=False,
                            )
                            nc.tensor.matmul(
                                zp[g][:, i], wl[:, kc, hsl], x_tiles(kc),
                                start=False, stop=(kc == nk - 1),
                            )

                cur = cursb.tile([128, TC, NST], F32, tag=cur_tag,
                                 name=f"{cur_tag}_{c}")
                for hc in range(HC):
                    g, i = hc // 2, hc % 2
                    for b in range(BL):
                        j = hc * BL + b
                        init = 0.0 if c == 0 else prev_cur[:, TC - 1, j:j + 1]
                        nc.vector.tensor_tensor_scan(
                            cur[:, :, j], c075, zp[g][:, i, b, :], init,
                            OP.mult, OP.add,
                        )

                vtmp = vtsb.tile([128, TC, NST], F32, tag=vt_tag,
                                 name=f"{vt_tag}_{c}")
                for t in range(TC):
                    nc.vector.scalar_tensor_tensor(
                        vtmp[:, t, :], vst, 1.0 - VOLT_DECAY, cur[:, t, :],
                        OP.mult, OP.add,
                    )
                    nc.vector.scalar_tensor_tensor(
                        vst, vtmp[:, t, :], THRESH, vtmp[:, t, :],
                        OP.is_lt, OP.mult,
                    )

                # spikes: fp16 operand for downstream GEMMs + fp32 output copy
                s16_view = s16[:, :, :, tsl].rearrange("p hc b t -> p t hc b")
                nc.vector.tensor_scalar(s16_view, vtmp, THRESH, None, OP.is_ge)
                sf = sfsb.tile([128, HC, BL, TC], F32, tag=sf_tag,
                               name=f"{sf_tag}_{c}")
                sf_view = sf.rearrange("p hc b t -> p t hc b")
                nc.vector.tensor_scalar(sf_view, vtmp, THRESH, None, OP.is_ge)

                # v staging: ACT transpose t-major -> (hc, b, t), then DMA
                vo = vosb.tile([128, HC, BL, TC], F32, tag=vo_tag,
                               name=f"{vo_tag}_{c}")
                nc.scalar.copy(vo, vtmp.rearrange("p t (hc b) -> p hc b t", hc=HC))
                for hc in range(HC):
                    nc.sync.dma_start(out=v_r[:, hc, :, tsl], in_=vo[:, hc])
                    nc.sync.dma_start(out=s_r[:, hc, :, tsl], in_=sf[:, hc])
                return cur

            prev_cur1 = None
            prev_cur2 = None
            for c in range(NCH):
                tsl = slice(c * TC, (c + 1) * TC)

                if c == 0:
                    x_c = x_first
                else:
                    x_c = []
                    for fc in range(FC):
                        x = xsb.tile([128, BL, TC], F16, tag=f"x{fc}",
                                     name=f"x{fc}_{c}")
                        nc.sync.dma_start(out=x, in_=spike_r[fc][:, :, tsl])
                        x_c.append(x)

                prev_cur1 = layer(
                    c, tsl, lambda kc: x_c[kc], w1h, w1l, FC, "cur1",
                    prev_cur1, v1st, "vt1", "vo1", "s1f", s1_16,
                    psum1, "g", s1_r, v1_r,
                )
                prev_cur2 = layer(
                    c, tsl, lambda kc: s1_16[:, kc, :, tsl], w2h, w2l, HC,
                    "cur2", prev_cur2, v2st, "vt2", "vo2", "s2f", s2_16,
                    psum2, "h", s2_r, v2_r,
                )

                # ---- readouts inline (PSUM tag g0 rotates with z1) ----
                rp = psum1.tile([128, 2, BL, TC], F32, tag="g0", name=f"rp_{c}")
                for kc in range(HC):
                    nc.tensor.matmul(rp[:, 0], r1h[:, kc], s1_16[:, kc, :, tsl],
                                     start=(kc == 0), stop=False)
                    nc.tensor.matmul(rp[:, 0], r1l[:, kc], s1_16[:, kc, :, tsl],
                                     start=False, stop=(kc == HC - 1))
                for kc in range(HC):
                    nc.tensor.matmul(rp[:, 1], r2h[:, kc], s2_16[:, kc, :, tsl],
                                     start=(kc == 0), stop=False)
                    nc.tensor.matmul(rp[:, 1], r2l[:, kc], s2_16[:, kc, :, tsl],
                                     start=False, stop=(kc == HC - 1))
                nc.scalar.copy(r1sb[:, :, tsl], rp[:, 0])
                nc.scalar.copy(r2sb[:, :, tsl], rp[:, 1])

            nc.sync.dma_start(out=r1_r, in_=r1sb)
            nc.sync.dma_start(out=r2_r, in_=r2sb)

    nc.compile()
    return nc


_NC_CACHE = {}


def _get_nc():
    if "nc" not in _NC_CACHE:
        _NC_CACHE["nc"] = build_nc()
    return _NC_CACHE["nc"]


def _split16(a):
    hi = a.astype(np.float16)
    lo = (a - hi.astype(np.float32)).astype(np.float16)
    return hi, lo


def run_cores(spike, W1, W2, R1, R2, trace=False):
    nc = _get_nc()
    w1hi, w1lo = _split16(np.ascontiguousarray(W1.T).astype(np.float32))
    w2hi, w2lo = _split16(np.ascontiguousarray(W2.T).astype(np.float32))
    r1hi, r1lo = _split16(np.ascontiguousarray(R1.T).astype(np.float32))
    r2hi, r2lo = _split16(np.ascontiguousarray(R2.T).astype(np.float32))
    x16 = np.ascontiguousarray(spike).astype(np.float16)
    in_maps = [
        {
            "x16": x16[c * BL:(c + 1) * BL],
            "w1hi": w1hi, "w1lo": w1lo,
            "w2hi": w2hi, "w2lo": w2lo,
            "r1hi": r1hi, "r1lo": r1lo,
            "r2hi": r2hi, "r2lo": r2lo,
        }
        for c in range(N_CORES)
    ]
    return run_bass_kernel_spmd(
        nc, in_maps, list(range(N_CORES)), trace=trace
    )


def kernel(spike, W1, W2, R1, R2):
    res = run_cores(spike, W1, W2, R1, R2).results
    s1 = np.concatenate([res[c]["s1"] for c in range(N_CORES)], axis=0)
    v1 = np.concatenate([res[c]["v1"] for c in range(N_CORES)], axis=0)
    s2 = np.concatenate([res[c]["s2"] for c in range(N_CORES)], axis=0)
    v2 = np.concatenate([res[c]["v2"] for c in range(N_CORES)], axis=0)
    r1 = np.concatenate([res[c]["r1"] for c in range(N_CORES)], axis=0)
    r2 = np.concatenate([res[c]["r2"] for c in range(N_CORES)], axis=0)
    c1 = np.float32(s1.mean(dtype=np.float64))
    c2 = np.float32(s2.mean(dtype=np.float64))
    return ((s1, s2), (r1, r2), (v1, v2), (c1, c2))


# revision 17
# speedup vs baseline: 1.0716x; 1.0716x over previous
"""DECOLLE network (2 CUBA-LIF layers + linear readouts) on 8 trn2 NeuronCores.

Sharding: data-parallel over batch (32 -> 4 per core), weights replicated.

v4 design:
- GEMMs in full fp32 (exact reference-class numerics; zero spike flips).
- Current recurrence via hardware tensor_tensor_scan along t, reading z
  from SBUF (ACT stages PSUM->SBUF) with contiguous data1.
- Voltage chain: ONE custom DVE op per step (CUBA_STEP_ANT):
      s'[t] = select(0.97*s'[t-1] + cur[t] < thresh, ..., 0)
  carrying only the post-reset state; the pre-reset voltage is recovered
  off-chain in bulk: v[t] = 0.97*s'[t-1] + cur[t] (in-place over cur).
- Spikes: one bulk is_ge per chunk-layer into a per-chunk fp32 ring that
  feeds the next layer's GEMM, the readouts, and the DMA out.
- ACT does psum staging, v-staging transposes, and readout copies.
- Readouts accumulated inline per chunk, PSUM bank shared with z1 rotation.
"""

import sys

sys.path.insert(0, "/opt/trn_rl_repo")

import numpy as np

import concourse.bass as bass
import concourse.tile as tile
from concourse import bacc, dve_ops, mybir
from concourse.bass_utils import run_bass_kernel_spmd
from concourse.dve_spec import C0, C1, Spec, Src0, Src1, Zero, lower, select
from concourse.dve_uop import DveOpSpec

F32 = mybir.dt.float32
OP = mybir.AluOpType

THRESH = 1.25
CUR_DECAY = 0.25
VOLT_DECAY = 0.03
VD = 1.0 - VOLT_DECAY

B, F, T = 32, 512, 256
H, OUT = 1024, 128
N_CORES = 8
BL = B // N_CORES          # 4 batches per core
TC = 64                    # time chunk
NCH = T // TC              # 4 chunks
HC = H // 128              # 8 h chunks
FC = F // 128              # 4 f chunks
NST = HC * BL              # 32 state lanes per partition-row


def _register_cuba_op():
    """One fused CUBA-LIF voltage step:
    out = x if x < c1 else 0, with x = in0*c0 + in1."""
    name = "CUBA_STEP_ANT"
    for op in dve_ops.OPS:
        if op.name == name:
            return op
    x = Src0 * C0 + Src1

    def _ref(in0, in1, c0, c1, c2):
        xx = in0 * c0 + in1
        return np.where(xx < c1, xx, 0.0)

    spec = Spec(body=select(x < C1, x, Zero), reference=_ref)
    opcode = dve_ops._CUSTOM_DVE_ROW_BASE + len(dve_ops.OPS)
    shas = {}
    for ver in ("v3", "v4"):
        try:
            uops = lower(spec, ver=ver)
            shas[ver] = DveOpSpec(
                name=name, opcode=opcode, uops=uops, rd1_en=True
            ).sha(ver)
        except Exception:
            pass
    op = dve_ops.DveOp(name, spec, subdim=False, uops_sha=shas)
    dve_ops.OPS.append(op)
    dve_ops.CUSTOM_DVE_SPECS[name] = spec
    dve_ops._SUB_OPCODE_FOR_NAME[name] = opcode
    return op


def build_nc():
    cuba_op = _register_cuba_op()
    nc = bacc.Bacc(None)

    spike = nc.declare_dram_parameter("spike", [BL, F, T], F32, isOutput=False)
    w1t = nc.declare_dram_parameter("W1T", [F, H], F32, isOutput=False)
    w2t = nc.declare_dram_parameter("W2T", [H, H], F32, isOutput=False)
    r1t = nc.declare_dram_parameter("R1T", [H, OUT], F32, isOutput=False)
    r2t = nc.declare_dram_parameter("R2T", [H, OUT], F32, isOutput=False)

    s1_d = nc.declare_dram_parameter("s1", [BL, H, T], F32, isOutput=True)
    v1_d = nc.declare_dram_parameter("v1", [BL, H, T], F32, isOutput=True)
    s2_d = nc.declare_dram_parameter("s2", [BL, H, T], F32, isOutput=True)
    v2_d = nc.declare_dram_parameter("v2", [BL, H, T], F32, isOutput=True)
    r1_d = nc.declare_dram_parameter("r1", [BL, OUT, T], F32, isOutput=True)
    r2_d = nc.declare_dram_parameter("r2", [BL, OUT, T], F32, isOutput=True)

    spike_r = spike[:].rearrange("b (fc p) t -> fc p b t", p=128)
    s1_r = s1_d[:].rearrange("b (hc p) t -> p hc b t", p=128)
    v1_r = v1_d[:].rearrange("b (hc p) t -> p hc b t", p=128)
    s2_r = s2_d[:].rearrange("b (hc p) t -> p hc b t", p=128)
    v2_r = v2_d[:].rearrange("b (hc p) t -> p hc b t", p=128)
    r1_r = r1_d[:].rearrange("b o t -> o b t")
    r2_r = r2_d[:].rearrange("b o t -> o b t")

    w1_r = w1t[:].rearrange("(fc p) h -> p fc h", p=128)
    w2_r = w2t[:].rearrange("(kc p) h -> p kc h", p=128)
    r1w_r = r1t[:].rearrange("(kc p) o -> p kc o", p=128)
    r2w_r = r2t[:].rearrange("(kc p) o -> p kc o", p=128)

    with tile.TileContext(nc) as tc:
        with (
            tc.tile_pool(name="wsb", bufs=1) as wsb,
            tc.tile_pool(name="xsb", bufs=2) as xsb,
            tc.tile_pool(name="zsb", bufs=1) as zsb,
            tc.tile_pool(name="cursb", bufs=2) as cursb,
            tc.tile_pool(name="spsb", bufs=2) as spsb,
            tc.tile_pool(name="cssb", bufs=2) as cssb,
            tc.tile_pool(name="vosb", bufs=1) as vosb,
            tc.tile_pool(name="ssb", bufs=2) as ssb,
            tc.tile_pool(name="rsb", bufs=1) as rsb,
            tc.tile_pool(name="psum1", bufs=1, space="PSUM") as psum1,
            tc.tile_pool(name="psum2", bufs=1, space="PSUM") as psum2,
        ):
            # x chunk 0 first on the SP ring so PE can start early;
            # weights go on the ACT ring (separate FIFO).
            x_first = []
            for fc in range(FC):
                x = xsb.tile([128, BL, TC], F32, tag=f"x{fc}", name=f"x{fc}_0")
                nc.sync.dma_start(out=x, in_=spike_r[fc][:, :, 0:TC])
                x_first.append(x)

            w1 = wsb.tile([128, FC, H], F32, name="w1")
            nc.scalar.dma_start(out=w1, in_=w1_r)
            w2 = wsb.tile([128, HC, H], F32, name="w2")
            nc.scalar.dma_start(out=w2, in_=w2_r)
            rw1 = wsb.tile([128, HC, OUT], F32, name="rw1")
            nc.scalar.dma_start(out=rw1, in_=r1w_r)
            rw2 = wsb.tile([128, HC, OUT], F32, name="rw2")
            nc.scalar.dma_start(out=rw2, in_=r2w_r)

            c075 = wsb.tile([128, TC], F32, name="c075")
            nc.vector.memset(c075, 1.0 - CUR_DECAY)
            zst = wsb.tile([128, 1, NST], F32, name="zst")
            nc.vector.memset(zst, 0.0)

            r1sb = rsb.tile([128, BL, T], F32, name="r1sb")
            r2sb = rsb.tile([128, BL, T], F32, name="r2sb")

            def layer(c, tsl, lay, rhs, w, nk, prev_csave, prev_sp,
                      ps, ps_tag, s_r, v_r):
                """One layer chunk. rhs(kc) -> [K=128, BL, TC] fp32 operand.
                Returns (csave, sp_ring, s_ring)."""
                zp = []
                for g in range(HC // 2):
                    p = ps.tile([128, 2, BL, TC], F32, tag=f"{ps_tag}{g}",
                                name=f"z{ps_tag}{g}_{c}")
                    zp.append(p)
                for g in range(HC // 2):
                    for i in range(2):
                        hc = 2 * g + i
                        hsl = slice(hc * 128, (hc + 1) * 128)
                        for kc in range(nk):
                            nc.tensor.matmul(
                                zp[g][:, i], w[:, kc, hsl], rhs(kc),
                                start=(kc == 0), stop=(kc == nk - 1),
                            )

                # stage z to SBUF (ACT) so the scans read SBUF contiguously
                zs = []
                for g in range(HC // 2):
                    z = zsb.tile([128, 2, BL, TC], F32, tag=f"z{lay}{g}",
                                 name=f"z{lay}{g}_{c}")
                    nc.scalar.copy(z, zp[g])
                    zs.append(z)

                # current scan into t-major ring [p, t, j], j = hc*BL + b
                cur = cursb.tile([128, TC, NST], F32, tag=f"cur{lay}",
                                 name=f"cur{lay}_{c}")
                for hc in range(HC):
                    g, i = hc // 2, hc % 2
                    for b in range(BL):
                        j = hc * BL + b
                        init = 0.0 if c == 0 else prev_csave[:, 0, j:j + 1]
                        nc.vector.tensor_tensor_scan(
                            cur[:, :, j], c075, zs[g][:, i, b, :], init,
                            OP.mult, OP.add,
                        )

                # save last cur column (bulk pass below overwrites cur)
                csave = cssb.tile([128, 1, NST], F32, tag=f"cs{lay}",
                                  name=f"cs{lay}_{c}")
                nc.vector.tensor_copy(csave, cur[:, TC - 1:TC, :])

                # voltage chain: one fused op per step
                sp = spsb.tile([128, TC, NST], F32, tag=f"sp{lay}",
                               name=f"sp{lay}_{c}")
                prev0 = zst if c == 0 else prev_sp[:, TC - 1:TC, :]
                nc.vector._custom_dve(
                    cuba_op, out=sp[:, 0:1, :], in0=prev0,
                    in1=cur[:, 0:1, :], s0=VD, s1=THRESH,
                )
                for t in range(1, TC):
                    nc.vector._custom_dve(
                        cuba_op, out=sp[:, t, :], in0=sp[:, t - 1, :],
                        in1=cur[:, t, :], s0=VD, s1=THRESH,
                    )

                # bulk pre-reset voltage recovery, in place over cur
                nc.vector.scalar_tensor_tensor(
                    cur[:, 0:1, :], prev0, VD, cur[:, 0:1, :],
                    OP.mult, OP.add,
                )
                nc.vector.scalar_tensor_tensor(
                    cur[:, 1:, :], sp[:, 0:TC - 1, :], VD, cur[:, 1:, :],
                    OP.mult, OP.add,
                )

                # spikes: one bulk op into the per-chunk fp32 ring
                # (feeds next layer GEMM + readout + DMA out)
                s_ring = ssb.tile([128, HC, BL, TC], F32, tag=f"s{lay}",
                                  name=f"s{lay}_{c}")
                s_view = s_ring.rearrange("p hc b t -> p t (hc b)")
                nc.vector.tensor_scalar(s_view, cur, THRESH, None, OP.is_ge)

                # v staging: ACT transpose t-major -> (hc, b, t), then DMA
                vo = vosb.tile([128, HC, BL, TC], F32, tag=f"vo{lay}",
                               name=f"vo{lay}_{c}")
                nc.scalar.copy(
                    vo, cur.rearrange("p t (hc b) -> p hc b t", hc=HC))
                for hc in range(HC):
                    nc.sync.dma_start(out=v_r[:, hc, :, tsl], in_=vo[:, hc])
                    nc.sync.dma_start(out=s_r[:, hc, :, tsl],
                                      in_=s_ring[:, hc])
                return csave, sp, s_ring

            prev_cs1 = prev_cs2 = None
            prev_sp1 = prev_sp2 = None
            for c in range(NCH):
                tsl = slice(c * TC, (c + 1) * TC)

                if c == 0:
                    x_c = x_first
                else:
                    x_c = []
                    for fc in range(FC):
                        x = xsb.tile([128, BL, TC], F32, tag=f"x{fc}",
                                     name=f"x{fc}_{c}")
                        nc.sync.dma_start(out=x, in_=spike_r[fc][:, :, tsl])
                        x_c.append(x)

                prev_cs1, prev_sp1, s1c = layer(
                    c, tsl, 1, lambda kc: x_c[kc], w1, FC,
                    prev_cs1, prev_sp1, psum1, "g", s1_r, v1_r,
                )
                prev_cs2, prev_sp2, s2c = layer(
                    c, tsl, 2, lambda kc: s1c[:, kc], w2, HC,
                    prev_cs2, prev_sp2, psum2, "h", s2_r, v2_r,
                )

                # readouts inline (PSUM tag g0 rotates with z1)
                rp = psum1.tile([128, 2, BL, TC], F32, tag="g0", name=f"rp_{c}")
                for kc in range(HC):
                    nc.tensor.matmul(rp[:, 0], rw1[:, kc], s1c[:, kc],
                                     start=(kc == 0), stop=(kc == HC - 1))
                for kc in range(HC):
                    nc.tensor.matmul(rp[:, 1], rw2[:, kc], s2c[:, kc],
                                     start=(kc == 0), stop=(kc == HC - 1))
                nc.scalar.copy(r1sb[:, :, tsl], rp[:, 0])
                nc.scalar.copy(r2sb[:, :, tsl], rp[:, 1])

            nc.sync.dma_start(out=r1_r, in_=r1sb)
            nc.sync.dma_start(out=r2_r, in_=r2sb)

    nc.compile()
    return nc


_NC_CACHE = {}


def _get_nc():
    if "nc" not in _NC_CACHE:
        _NC_CACHE["nc"] = build_nc()
    return _NC_CACHE["nc"]


def run_cores(spike, W1, W2, R1, R2, trace=False):
    nc = _get_nc()
    w1t = np.ascontiguousarray(W1.T).astype(np.float32)
    w2t = np.ascontiguousarray(W2.T).astype(np.float32)
    r1t = np.ascontiguousarray(R1.T).astype(np.float32)
    r2t = np.ascontiguousarray(R2.T).astype(np.float32)
    spike = np.ascontiguousarray(spike).astype(np.float32)
    in_maps = [
        {
            "spike": spike[c * BL:(c + 1) * BL],
            "W1T": w1t,
            "W2T": w2t,
            "R1T": r1t,
            "R2T": r2t,
        }
        for c in range(N_CORES)
    ]
    return run_bass_kernel_spmd(
        nc, in_maps, list(range(N_CORES)), trace=trace
    )


def kernel(spike, W1, W2, R1, R2):
    res = run_cores(spike, W1, W2, R1, R2).results
    s1 = np.concatenate([res[c]["s1"] for c in range(N_CORES)], axis=0)
    v1 = np.concatenate([res[c]["v1"] for c in range(N_CORES)], axis=0)
    s2 = np.concatenate([res[c]["s2"] for c in range(N_CORES)], axis=0)
    v2 = np.concatenate([res[c]["v2"] for c in range(N_CORES)], axis=0)
    r1 = np.concatenate([res[c]["r1"] for c in range(N_CORES)], axis=0)
    r2 = np.concatenate([res[c]["r2"] for c in range(N_CORES)], axis=0)
    c1 = np.float32(s1.mean(dtype=np.float64))
    c2 = np.float32(s2.mean(dtype=np.float64))
    return ((s1, s2), (r1, r2), (v1, v2), (c1, c2))


# revision 20
# speedup vs baseline: 1.5402x; 1.4373x over previous
"""DECOLLE network (2 CUBA-LIF layers + linear readouts) on 8 trn2 NeuronCores.

Sharding: data-parallel over batch (32 -> 4 per core), weights replicated.

v4 design:
- GEMMs in full fp32 (exact reference-class numerics; zero spike flips).
- Current recurrence via hardware tensor_tensor_scan along t, reading z
  from SBUF (ACT stages PSUM->SBUF) with contiguous data1.
- Voltage chain: ONE custom DVE op per step (CUBA_STEP_ANT):
      s'[t] = select(0.97*s'[t-1] + cur[t] < thresh, ..., 0)
  carrying only the post-reset state; the pre-reset voltage is recovered
  off-chain in bulk: v[t] = 0.97*s'[t-1] + cur[t] (in-place over cur).
- Spikes: one bulk is_ge per chunk-layer into a per-chunk fp32 ring that
  feeds the next layer's GEMM, the readouts, and the DMA out.
- ACT does psum staging, v-staging transposes, and readout copies.
- Readouts accumulated inline per chunk, PSUM bank shared with z1 rotation.
"""

import sys

sys.path.insert(0, "/opt/trn_rl_repo")

import numpy as np

import concourse.bass as bass
import concourse.tile as tile
from concourse import bacc, dve_ops, mybir
from concourse.bass_utils import run_bass_kernel_spmd
from concourse.dve_spec import C0, C1, Spec, Src0, Src1, Zero, lower, select
from concourse.dve_uop import DveOpSpec

F32 = mybir.dt.float32
OP = mybir.AluOpType

THRESH = 1.25
CUR_DECAY = 0.25
VOLT_DECAY = 0.03
VD = 1.0 - VOLT_DECAY

B, F, T = 32, 512, 256
H, OUT = 1024, 128
N_CORES = 8
BL = B // N_CORES          # 4 batches per core
TC = 64                    # time chunk
NCH = T // TC              # 4 chunks
HC = H // 128              # 8 h chunks
FC = F // 128              # 4 f chunks
NST = HC * BL              # 32 state lanes per partition-row


def _register_cuba_op():
    """One fused CUBA-LIF voltage step:
    out = x if x < c1 else 0, with x = in0*c0 + in1."""
    name = "CUBA_STEP_ANT"
    for op in dve_ops.OPS:
        if op.name == name:
            return op
    x = Src0 * C0 + Src1

    def _ref(in0, in1, c0, c1, c2):
        xx = in0 * c0 + in1
        return np.where(xx < c1, xx, 0.0)

    spec = Spec(body=select(x < C1, x, Zero), reference=_ref)
    opcode = dve_ops._CUSTOM_DVE_ROW_BASE + len(dve_ops.OPS)
    shas = {}
    for ver in ("v3", "v4"):
        try:
            uops = lower(spec, ver=ver)
            shas[ver] = DveOpSpec(
                name=name, opcode=opcode, uops=uops, rd1_en=True
            ).sha(ver)
        except Exception:
            pass
    op = dve_ops.DveOp(name, spec, subdim=False, uops_sha=shas)
    dve_ops.OPS.append(op)
    dve_ops.CUSTOM_DVE_SPECS[name] = spec
    dve_ops._SUB_OPCODE_FOR_NAME[name] = opcode
    return op


def build_nc():
    cuba_op = _register_cuba_op()
    nc = bacc.Bacc(None)

    spike = nc.declare_dram_parameter("spike", [BL, F, T], F32, isOutput=False)
    w1t = nc.declare_dram_parameter("W1T", [F, H], F32, isOutput=False)
    w2t = nc.declare_dram_parameter("W2T", [H, H], F32, isOutput=False)
    r1t = nc.declare_dram_parameter("R1T", [H, OUT], F32, isOutput=False)
    r2t = nc.declare_dram_parameter("R2T", [H, OUT], F32, isOutput=False)

    s1_d = nc.declare_dram_parameter("s1", [BL, H, T], F32, isOutput=True)
    v1_d = nc.declare_dram_parameter("v1", [BL, H, T], F32, isOutput=True)
    s2_d = nc.declare_dram_parameter("s2", [BL, H, T], F32, isOutput=True)
    v2_d = nc.declare_dram_parameter("v2", [BL, H, T], F32, isOutput=True)
    r1_d = nc.declare_dram_parameter("r1", [BL, OUT, T], F32, isOutput=True)
    r2_d = nc.declare_dram_parameter("r2", [BL, OUT, T], F32, isOutput=True)

    spike_r = spike[:].rearrange("b (fc p) t -> fc p b t", p=128)
    s1_r = s1_d[:].rearrange("b (hc p) t -> p hc b t", p=128)
    v1_r = v1_d[:].rearrange("b (hc p) t -> p hc b t", p=128)
    s2_r = s2_d[:].rearrange("b (hc p) t -> p hc b t", p=128)
    v2_r = v2_d[:].rearrange("b (hc p) t -> p hc b t", p=128)
    r1_r = r1_d[:].rearrange("b o t -> o b t")
    r2_r = r2_d[:].rearrange("b o t -> o b t")

    w1_r = w1t[:].rearrange("(fc p) h -> p fc h", p=128)
    w2_r = w2t[:].rearrange("(kc p) h -> p kc h", p=128)
    r1w_r = r1t[:].rearrange("(kc p) o -> p kc o", p=128)
    r2w_r = r2t[:].rearrange("(kc p) o -> p kc o", p=128)

    with tile.TileContext(nc) as tc:
        with (
            tc.tile_pool(name="wsb", bufs=1) as wsb,
            tc.tile_pool(name="xsb", bufs=2) as xsb,
            tc.tile_pool(name="zsb", bufs=1) as zsb,
            tc.tile_pool(name="cursb", bufs=2) as cursb,
            tc.tile_pool(name="spsb", bufs=2) as spsb,
            tc.tile_pool(name="cssb", bufs=2) as cssb,
            tc.tile_pool(name="vosb", bufs=1) as vosb,
            tc.tile_pool(name="ssb", bufs=2) as ssb,
            tc.tile_pool(name="rsb", bufs=1) as rsb,
            tc.tile_pool(name="psum1", bufs=1, space="PSUM") as psum1,
            tc.tile_pool(name="psum2", bufs=1, space="PSUM") as psum2,
        ):
            # x chunk 0 first on the SP ring so PE can start early;
            # weights go on the ACT ring (separate FIFO).
            x_first = []
            for fc in range(FC):
                x = xsb.tile([128, BL, TC], F32, tag=f"x{fc}", name=f"x{fc}_0")
                nc.scalar.dma_start(out=x, in_=spike_r[fc][:, :, 0:TC])
                x_first.append(x)

            w1 = wsb.tile([128, FC, H], F32, name="w1")
            nc.scalar.dma_start(out=w1, in_=w1_r)
            w2 = wsb.tile([128, HC, H], F32, name="w2")
            nc.scalar.dma_start(out=w2, in_=w2_r)
            rw1 = wsb.tile([128, HC, OUT], F32, name="rw1")
            nc.scalar.dma_start(out=rw1, in_=r1w_r)
            rw2 = wsb.tile([128, HC, OUT], F32, name="rw2")
            nc.scalar.dma_start(out=rw2, in_=r2w_r)

            c075 = wsb.tile([128, TC], F32, name="c075")
            nc.vector.memset(c075, 1.0 - CUR_DECAY)
            zst = wsb.tile([128, 1, NST], F32, name="zst")
            nc.vector.memset(zst, 0.0)

            r1sb = rsb.tile([128, BL, T], F32, name="r1sb")
            r2sb = rsb.tile([128, BL, T], F32, name="r2sb")

            def l_gemm(c, lay, rhs, w, nk, ps, ps_tag):
                """GEMM for one layer chunk into PSUM, then ACT-stage to SBUF.
                Returns the staged SBUF z tiles."""
                zp = []
                for g in range(HC // 2):
                    p = ps.tile([128, 2, BL, TC], F32, tag=f"{ps_tag}{g}",
                                name=f"z{ps_tag}{g}_{c}")
                    zp.append(p)
                for g in range(HC // 2):
                    for i in range(2):
                        hc = 2 * g + i
                        hsl = slice(hc * 128, (hc + 1) * 128)
                        for kc in range(nk):
                            nc.tensor.matmul(
                                zp[g][:, i], w[:, kc, hsl], rhs(kc),
                                start=(kc == 0), stop=(kc == nk - 1),
                            )
                zs = []
                for g in range(HC // 2):
                    z = zsb.tile([128, 2, BL, TC], F32, tag=f"z{lay}{g}",
                                 name=f"z{lay}{g}_{c}")
                    nc.scalar.copy(z, zp[g])
                    zs.append(z)
                return zs

            def l_rest(c, tsl, lay, zs, prev_csave, prev_sp, s_r, v_r):
                """Scan + voltage chain + spikes + staging/DMA for one chunk.
                Returns (csave, sp_ring, s_ring)."""
                # current scan into t-major ring [p, t, j], j = hc*BL + b
                cur = cursb.tile([128, TC, NST], F32, tag=f"cur{lay}",
                                 name=f"cur{lay}_{c}")
                for hc in range(HC):
                    g, i = hc // 2, hc % 2
                    for b in range(BL):
                        j = hc * BL + b
                        init = 0.0 if c == 0 else prev_csave[:, 0, j:j + 1]
                        nc.vector.tensor_tensor_scan(
                            cur[:, :, j], c075, zs[g][:, i, b, :], init,
                            OP.mult, OP.add,
                        )

                # save last cur column (bulk pass below overwrites cur)
                csave = cssb.tile([128, 1, NST], F32, tag=f"cs{lay}",
                                  name=f"cs{lay}_{c}")
                nc.vector.tensor_copy(csave, cur[:, TC - 1:TC, :])

                # voltage chain: one fused op per step
                sp = spsb.tile([128, TC, NST], F32, tag=f"sp{lay}",
                               name=f"sp{lay}_{c}")
                prev0 = zst if c == 0 else prev_sp[:, TC - 1:TC, :]
                nc.vector._custom_dve(
                    cuba_op, out=sp[:, 0:1, :], in0=prev0,
                    in1=cur[:, 0:1, :], s0=VD, s1=THRESH,
                )
                for t in range(1, TC):
                    nc.vector._custom_dve(
                        cuba_op, out=sp[:, t, :], in0=sp[:, t - 1, :],
                        in1=cur[:, t, :], s0=VD, s1=THRESH,
                    )

                # bulk pre-reset voltage recovery, in place over cur
                nc.vector.scalar_tensor_tensor(
                    cur[:, 0:1, :], prev0, VD, cur[:, 0:1, :],
                    OP.mult, OP.add,
                )
                nc.vector.scalar_tensor_tensor(
                    cur[:, 1:, :], sp[:, 0:TC - 1, :], VD, cur[:, 1:, :],
                    OP.mult, OP.add,
                )

                # spikes: one bulk op into the per-chunk fp32 ring
                # (feeds next layer GEMM + readout + DMA out)
                s_ring = ssb.tile([128, HC, BL, TC], F32, tag=f"s{lay}",
                                  name=f"s{lay}_{c}")
                s_view = s_ring.rearrange("p hc b t -> p t (hc b)")
                nc.vector.tensor_scalar(s_view, cur, THRESH, None, OP.is_ge)

                # v staging: ACT transpose t-major -> (hc, b, t), then DMA
                vo = vosb.tile([128, HC, BL, TC], F32, tag=f"vo{lay}",
                               name=f"vo{lay}_{c}")
                nc.scalar.copy(
                    vo, cur.rearrange("p t (hc b) -> p hc b t", hc=HC))
                for hc in range(HC):
                    nc.sync.dma_start(out=v_r[:, hc, :, tsl], in_=vo[:, hc])
                    nc.sync.dma_start(out=s_r[:, hc, :, tsl],
                                      in_=s_ring[:, hc])
                return csave, sp, s_ring

            def readouts(c, s1c, s2c):
                tsl = slice(c * TC, (c + 1) * TC)
                rp = psum1.tile([128, 2, BL, TC], F32, tag="g0",
                                name=f"rp_{c}")
                for kc in range(HC):
                    nc.tensor.matmul(rp[:, 0], rw1[:, kc], s1c[:, kc],
                                     start=(kc == 0), stop=(kc == HC - 1))
                for kc in range(HC):
                    nc.tensor.matmul(rp[:, 1], rw2[:, kc], s2c[:, kc],
                                     start=(kc == 0), stop=(kc == HC - 1))
                nc.scalar.copy(r1sb[:, :, tsl], rp[:, 0])
                nc.scalar.copy(r2sb[:, :, tsl], rp[:, 1])

            # software pipeline: PE runs z1(c+1) and readouts(c-1) while DVE
            # works through chunk c's scans/voltage chain.
            prev_cs1 = prev_cs2 = None
            prev_sp1 = prev_sp2 = None
            s_hist = {}
            zs1 = l_gemm(0, 1, lambda kc: x_first[kc], w1, FC, psum1, "g")
            for c in range(NCH):
                tsl = slice(c * TC, (c + 1) * TC)

                prev_cs1, prev_sp1, s1c = l_rest(
                    c, tsl, 1, zs1, prev_cs1, prev_sp1, s1_r, v1_r)

                if c + 1 < NCH:
                    x_c = []
                    nsl = slice((c + 1) * TC, (c + 2) * TC)
                    for fc in range(FC):
                        x = xsb.tile([128, BL, TC], F32, tag=f"x{fc}",
                                     name=f"x{fc}_{c + 1}")
                        nc.scalar.dma_start(out=x, in_=spike_r[fc][:, :, nsl])
                        x_c.append(x)
                    zs1 = l_gemm(c + 1, 1, lambda kc: x_c[kc], w1, FC,
                                 psum1, "g")

                if c >= 1:
                    readouts(c - 1, *s_hist[c - 1])

                zs2 = l_gemm(c, 2, lambda kc: s1c[:, kc], w2, HC, psum2, "h")
                prev_cs2, prev_sp2, s2c = l_rest(
                    c, tsl, 2, zs2, prev_cs2, prev_sp2, s2_r, v2_r)
                s_hist[c] = (s1c, s2c)

            readouts(NCH - 1, *s_hist[NCH - 1])
            nc.sync.dma_start(out=r1_r, in_=r1sb)
            nc.sync.dma_start(out=r2_r, in_=r2sb)

    nc.compile()
    return nc


_NC_CACHE = {}


def _get_nc():
    if "nc" not in _NC_CACHE:
        _NC_CACHE["nc"] = build_nc()
    return _NC_CACHE["nc"]


def run_cores(spike, W1, W2, R1, R2, trace=False):
    nc = _get_nc()
    w1t = np.ascontiguousarray(W1.T).astype(np.float32)
    w2t = np.ascontiguousarray(W2.T).astype(np.float32)
    r1t = np.ascontiguousarray(R1.T).astype(np.float32)
    r2t = np.ascontiguousarray(R2.T).astype(np.float32)
    spike = np.ascontiguousarray(spike).astype(np.float32)
    in_maps = [
        {
            "spike": spike[c * BL:(c + 1) * BL],
            "W1T": w1t,
            "W2T": w2t,
            "R1T": r1t,
            "R2T": r2t,
        }
        for c in range(N_CORES)
    ]
    return run_bass_kernel_spmd(
        nc, in_maps, list(range(N_CORES)), trace=trace
    )


def kernel(spike, W1, W2, R1, R2):
    res = run_cores(spike, W1, W2, R1, R2).results
    s1 = np.concatenate([res[c]["s1"] for c in range(N_CORES)], axis=0)
    v1 = np.concatenate([res[c]["v1"] for c in range(N_CORES)], axis=0)
    s2 = np.concatenate([res[c]["s2"] for c in range(N_CORES)], axis=0)
    v2 = np.concatenate([res[c]["v2"] for c in range(N_CORES)], axis=0)
    r1 = np.concatenate([res[c]["r1"] for c in range(N_CORES)], axis=0)
    r2 = np.concatenate([res[c]["r2"] for c in range(N_CORES)], axis=0)
    c1 = np.float32(s1.mean(dtype=np.float64))
    c2 = np.float32(s2.mean(dtype=np.float64))
    return ((s1, s2), (r1, r2), (v1, v2), (c1, c2))
